# revision 18
# baseline (speedup 1.0000x reference)
"""Multi-head causal attention (B=2, S=2048, H=16, DH=64, D=1024) on 8 TRN2 cores.

Sharding: Megatron tensor-parallel over heads — core c owns heads {2c, 2c+1}:
  * column-slices of Wq/Wk/Wv (128 cols each) + bias slices,
  * row-slice of Wo (128 rows),
  * full hidden_states (pre-transposed on host to [D, B*S]).
Each core computes a partial output (its 2 heads through Wo rows); host sums
the 8 partials (row-parallel unshard) — bo is added on device by core 0.

Device dataflow per core (all matmuls in float32r — full-rate fp32 with
15-bit mantissa — contract: every matmul operand must be produced by a
"rounding" instruction or DMA-ed from a float32r DRAM tensor):
  A) QKV: qT/kT/vT [128, 4096] = W_slice.T @ hiddenT  (contraction over D in
     8 chunks of 128; biases are per-partition adds during PSUM evacuation).
     vT is PE-transposed into V_aug [tok128, chunk, head, 65] with a ones
     column (65th) so the AV matmul also emits softmax denominators.
  B) Attention per (batch b, 256-query block qi), both heads:
     scoresT[kv,q] = kT_slice.T @ qT_slice  (K=64, N=256, two heads packed on
     PE row-groups 0-63/64-127), exp via ACT (scale=1/8) straight off PSUM,
     causal diag masked by a triangular multiply (even chunk) / restricting
     the AV matmul columns (odd chunk), then ctxT_aug[65, q] += V_aug.T@expT.
     Softmax denominators (row 64) are reciprocal-ed and broadcast across
     64 partitions with a K=1 ones matmul, then ctxT normalized via DVE.
  C) Output proj: outT[n,tok] = Wo_slice_chunk.T @ ctxT (+ bo via K=1 ones
     matmul, nonzero on core 0 only), DMA-ed to DRAM straight from PSUM.
"""
import os
import sys

sys.path.insert(0, "/opt/trn_rl_repo")

from contextlib import ExitStack

import numpy as np

import concourse.bass as bass
import concourse.mybir as mybir
import concourse.tile as tile
from concourse import bacc
from concourse.bass_utils import run_bass_kernel_spmd

F32 = mybir.dt.float32
F32R = mybir.dt.float32r
F16 = mybir.dt.float16
MM_DT = F32R if os.environ.get("KERNEL_F32R") == "1" else F16
MM_NP = np.float32 if os.environ.get("KERNEL_F32R") == "1" else np.float16

B, S, H, DH = 2, 2048, 16, 64
D = H * DH            # 1024
T = B * S             # 4096 tokens
NCORES = 8
HPC = H // NCORES     # 2 heads per core
KC = D // 128         # 8 contraction chunks
NG = T // 512         # 8 token groups for QKV
NQB = S // 256        # 8 query blocks per batch
NKV = T // 128        # 32 kv chunks of 128 tokens
EXPFN = mybir.ActivationFunctionType.Exp


def _body(nc, tc, ctx, t_in, t_out, t_out_dbg=None):
    xt, wq, wk, wv, wo, bq, bk, bv, bo, tri, ident, vones = t_in
    po = t_out

    const = ctx.enter_context(tc.tile_pool(name="const", bufs=1))
    big = ctx.enter_context(tc.tile_pool(name="big", bufs=1))
    xtp = ctx.enter_context(tc.tile_pool(name="xtp", bufs=3))
    vtp = ctx.enter_context(tc.tile_pool(name="vtp", bufs=2))
    ep = ctx.enter_context(tc.tile_pool(name="ep", bufs=6))
    rp = ctx.enter_context(tc.tile_pool(name="rp", bufs=2))
    rbp = ctx.enter_context(tc.tile_pool(name="rbp", bufs=2))

    psA = ctx.enter_context(tc.tile_pool(name="psA", bufs=2, space="PSUM"))
    psS = ctx.enter_context(tc.tile_pool(name="psS", bufs=2, space="PSUM"))
    psC = ctx.enter_context(tc.tile_pool(name="psC", bufs=1, space="PSUM"))

    # ---- constants / weights in SBUF
    wq_s = const.tile([128, KC, 128], MM_DT, tag="wq")
    wk_s = const.tile([128, KC, 128], MM_DT, tag="wk")
    wv_s = const.tile([128, KC, 128], MM_DT, tag="wv")
    wo_s = const.tile([128, D], MM_DT, tag="wo")
    bq_s = const.tile([128, 1], F32, tag="bq")
    bk_s = const.tile([128, 1], F32, tag="bk")
    bv_s = const.tile([128, 1], F32, tag="bv")
    bo_s = const.tile([128, KC], F32, tag="bo")
    tri_s = const.tile([128, 128], MM_DT, tag="tri")
    id_s = const.tile([128, 128], MM_DT, tag="ident")
    nc.sync.dma_start(wq_s[:], wq[:])
    nc.sync.dma_start(wk_s[:], wk[:])
    nc.sync.dma_start(wv_s[:], wv[:])
    nc.sync.dma_start(bq_s[:], bq[:])
    nc.sync.dma_start(bk_s[:], bk[:])
    nc.sync.dma_start(bv_s[:], bv[:])
    nc.gpsimd.dma_start(wo_s[:], wo[:])
    nc.gpsimd.dma_start(bo_s[:], bo[:])
    nc.gpsimd.dma_start(tri_s[:], tri[:])
    nc.gpsimd.dma_start(id_s[:], ident[:])

    qT = big.tile([128, T], MM_DT, tag="qT")
    kT = big.tile([128, T], MM_DT, tag="kT")
    vT = big.tile([128, T], MM_DT, tag="vT")
    ctxT = big.tile([128, T], MM_DT, tag="ctxT")
    vaug = big.tile([128, NKV, HPC, 65], MM_DT, tag="vaug")
    # ones column of V_aug (softmax denominator trick)
    nc.gpsimd.dma_start(vaug[:, :, :, 64].rearrange("p c h -> p (c h)"), vones[:])


    if t_out_dbg is not None:
        dbgp = ctx.enter_context(tc.tile_pool(name="dbgp", bufs=1))
        dbg_craw_s = dbgp.tile([128, T], F32, tag="craw")
        dbg_den_s = dbgp.tile([128, T], F32, tag="den")

    woc = wo_s[:].rearrange("p (c n) -> p c n", c=KC)
    osp = ctx.enter_context(tc.tile_pool(name="osp", bufs=6))

    # ---- fused streaming loop: per 512-token group, QKV -> transposes ->
    # attention for the two completed 256-query blocks -> output projection
    for g in range(NG):
        cols = slice(g * 512, (g + 1) * 512)
        xg = xtp.tile([128, KC, 512], MM_DT, tag="xt")
        nc.sync.dma_start(xg[:], xt[g])

        for (w_s, b_s, dst) in ((wq_s, bq_s, qT), (wk_s, bk_s, kT)):
            acc = psA.tile([128, 512], F32, tag="pqkv")
            for k in range(KC):
                nc.tensor.matmul(acc[:], w_s[:, k, :], xg[:, k, :],
                                 start=(k == 0), stop=(k == KC - 1))
            nc.vector.tensor_scalar_add(dst[:, cols], acc[:], b_s[:])

        acc = psA.tile([128, 512], F32, tag="pqkv")
        for k in range(KC):
            nc.tensor.matmul(acc[:], wv_s[:, k, :], xg[:, k, :],
                             start=(k == 0), stop=(k == KC - 1))
        nc.vector.tensor_scalar_add(vT[:, cols], acc[:], bv_s[:])

        for t4 in range(4):
            chunk = g * 4 + t4
            tp = psA.tile([128, 128], MM_DT, tag="pqkv")
            nc.tensor.transpose(tp[:], vT[:, chunk * 128:(chunk + 1) * 128],
                                id_s[:])
            nc.vector.tensor_copy(
                vaug[:, chunk, :, 0:64],
                tp[:].rearrange("p (h d) -> p h d", h=HPC))

        b = g // (S // 512)
        for qi in ((g % (S // 512)) * 2, (g % (S // 512)) * 2 + 1):
            qcols = slice(b * S + qi * 256, b * S + qi * 256 + 256)
            nch = 2 * (qi + 1)          # kv chunks of 128 for this q block
            ct0 = psC.tile([65, 256], F32, tag="ct0")
            ct1 = psC.tile([65, 256], F32, tag="ct1")
            cts = [ct0, ct1]
            nwaves = (nch + 1) // 2
            for w in range(nwaves):
                js = [j for j in (2 * w, 2 * w + 1) if j < nch]
                sp = psS.tile([128, 2 * HPC, 256], F32, tag="sp")
                for h in range(HPC):
                    for i, j in enumerate(js):
                        kcols = slice(b * S + j * 128, b * S + j * 128 + 128)
                        nc.tensor.matmul(
                            sp[:, 2 * h + i, :],
                            kT[h * 64:(h + 1) * 64, kcols],
                            qT[h * 64:(h + 1) * 64, qcols],
                            start=True, stop=True)
                e = ep.tile([128, 2 * HPC, 256], MM_DT, tag="e")
                nc.scalar.activation(e[:], sp[:], EXPFN, scale=0.125)
                for h in range(HPC):
                    for i, j in enumerate(js):
                        lhsT = vaug[:, b * (S // 128) + j, h, :]
                        first = (j == 0)
                        last = (j == nch - 1)
                        if j == nch - 2:  # even diag chunk: mask lower tri
                            nc.vector.tensor_mul(
                                e[:, 2 * h + i, 0:128],
                                e[:, 2 * h + i, 0:128], tri_s[:])
                            nc.tensor.matmul(cts[h][:, :], lhsT,
                                             e[:, 2 * h + i, :],
                                             start=first, stop=last)
                        elif j == nch - 1:  # odd diag: q first half all masked
                            nc.vector.tensor_mul(
                                e[:, 2 * h + i, 128:256],
                                e[:, 2 * h + i, 128:256], tri_s[:])
                            nc.tensor.matmul(cts[h][:, 128:256], lhsT,
                                             e[:, 2 * h + i, 128:256],
                                             start=first, stop=last)
                        else:
                            nc.tensor.matmul(cts[h][:, :], lhsT,
                                             e[:, 2 * h + i, :],
                                             start=first, stop=last)
            # normalize: ctxT[, q] = ct[0:64] * (1 / ct[64]) broadcast
            for h in range(HPC):
                sums = rp.tile([1, 256], F32, tag="sums")
                nc.vector.tensor_copy(sums[:], cts[h][64:65, :])
                r = rp.tile([1, 256], F32, tag="r")
                nc.vector.reciprocal_approx_fast(r[:], sums[:])
                rb = rbp.tile([64, 256], F32, tag="rb")
                nc.gpsimd.partition_broadcast(rb[:], r[:])
                nc.vector.tensor_mul(ctxT[h * 64:(h + 1) * 64, qcols],
                                     cts[h][0:64, :], rb[:])
                if t_out_dbg is not None:
                    nc.vector.tensor_copy(
                        dbg_craw_s[h * 64:(h + 1) * 64, qcols],
                        cts[h][0:64, :])
                    nc.vector.tensor_copy(
                        dbg_den_s[h * 64:(h + 1) * 64, qcols], rb[:])

            if qi % 2 == 1:
                # output projection for the completed 512-token block
                t = b * (S // 512) + qi // 2
                tcols = slice(t * 512, (t + 1) * 512)
                for c in range(KC):
                    op = psA.tile([128, 512], F32, tag="pqkv")
                    nc.tensor.matmul(op[:], woc[:, c, :], ctxT[:, tcols],
                                     start=True, stop=True)
                    ost = osp.tile([128, 512], F32, tag="ost")
                    if c % 2 == 0:
                        nc.vector.tensor_scalar_add(ost[:], op[:],
                                                    bo_s[:, c:c + 1])
                    else:
                        nc.scalar.activation(
                            ost[:], op[:],
                            mybir.ActivationFunctionType.Identity,
                            bias=bo_s[:, c:c + 1])
                    nc.sync.dma_start(po[c, :, tcols], ost[:])

    # ---- phase C: output projection (transposed out), bias, DMA from PSUM
    if t_out_dbg is not None:
        dq, dk, dc, dv, dcr, dde = t_out_dbg
        st = ctx.enter_context(tc.tile_pool(name="dbg", bufs=1))
        for src, dst in ((qT, dq), (kT, dk), (ctxT, dc)):
            tmp = st.tile([128, T], F32, tag="dbgt")
            nc.vector.tensor_copy(tmp[:], src[:])
            nc.sync.dma_start(dst[:], tmp[:])
        nc.sync.dma_start(dcr[:], dbg_craw_s[:])
        nc.sync.dma_start(dde[:], dbg_den_s[:])
        tmpv = st.tile([128, NKV * HPC * 65], F32, tag="dbgt")
        nc.vector.tensor_copy(
            tmpv[:], vaug[:].rearrange("p c h x -> p (c h x)"))
        nc.sync.dma_start(dv[:], tmpv[:])



_NC = None


def _build():
    global _NC
    if _NC is not None:
        return _NC
    nc = bacc.Bacc("TRN2", target_bir_lowering=False, debug=False,
                   num_devices=NCORES)
    t_in = [
        nc.dram_tensor("xt", [NG, 128, KC, 512], MM_DT, kind="ExternalInput").ap(),
        nc.dram_tensor("wq", [128, KC, 128], MM_DT, kind="ExternalInput").ap(),
        nc.dram_tensor("wk", [128, KC, 128], MM_DT, kind="ExternalInput").ap(),
        nc.dram_tensor("wv", [128, KC, 128], MM_DT, kind="ExternalInput").ap(),
        nc.dram_tensor("wo", [128, D], MM_DT, kind="ExternalInput").ap(),
        nc.dram_tensor("bq", [128, 1], F32, kind="ExternalInput").ap(),
        nc.dram_tensor("bk", [128, 1], F32, kind="ExternalInput").ap(),
        nc.dram_tensor("bv", [128, 1], F32, kind="ExternalInput").ap(),
        nc.dram_tensor("bo", [128, KC], F32, kind="ExternalInput").ap(),
        nc.dram_tensor("tri", [128, 128], MM_DT, kind="ExternalInput").ap(),
        nc.dram_tensor("ident", [128, 128], MM_DT, kind="ExternalInput").ap(),
        nc.dram_tensor("vones", [128, NKV * HPC], MM_DT,
                       kind="ExternalInput").ap(),
    ]
    po = nc.dram_tensor("po", [KC, 128, T], F32, kind="ExternalOutput").ap()
    t_out_dbg = None
    if os.environ.get("KERNEL_DEBUG_TAPS") == "1":
        t_out_dbg = [
            nc.dram_tensor("dbg_qT", [128, T], F32, kind="ExternalOutput").ap(),
            nc.dram_tensor("dbg_kT", [128, T], F32, kind="ExternalOutput").ap(),
            nc.dram_tensor("dbg_ctxT", [128, T], F32, kind="ExternalOutput").ap(),
            nc.dram_tensor("dbg_vaug", [128, NKV * HPC * 65], F32,
                           kind="ExternalOutput").ap(),
            nc.dram_tensor("dbg_craw", [128, T], F32, kind="ExternalOutput").ap(),
            nc.dram_tensor("dbg_den", [128, T], F32, kind="ExternalOutput").ap(),
        ]
    with tile.TileContext(nc) as tc, ExitStack() as ctx:
        _body(nc, tc, ctx, t_in, po, t_out_dbg)
    nc.compile()
    _NC = nc
    return nc


def _in_maps(hidden_states, Wq, bq, Wk, bk, Wv, bv, Wo, bo):
    hid = np.asarray(hidden_states, dtype=np.float32).reshape(T, D)
    hidT = hid.T.astype(MM_NP)                       # [D, T]
    xt = np.ascontiguousarray(
        hidT.reshape(KC, 128, NG, 512).transpose(2, 1, 0, 3))
    common = {
        "xt": xt,
        "tri": np.triu(np.ones((128, 128), MM_NP)),
        "ident": np.eye(128, dtype=MM_NP),
        "vones": np.ones((128, NKV * HPC), MM_NP),
    }
    maps = []
    for c in range(NCORES):
        cs = slice(c * 128, (c + 1) * 128)
        maps.append(dict(
            common,
            wq=np.ascontiguousarray(np.asarray(Wq)[:, cs].astype(MM_NP).reshape(KC, 128, 128).transpose(1, 0, 2)),
            wk=np.ascontiguousarray(np.asarray(Wk)[:, cs].astype(MM_NP).reshape(KC, 128, 128).transpose(1, 0, 2)),
            wv=np.ascontiguousarray(np.asarray(Wv)[:, cs].astype(MM_NP).reshape(KC, 128, 128).transpose(1, 0, 2)),
            wo=np.ascontiguousarray(np.asarray(Wo)[cs, :].astype(MM_NP)),
            bq=np.asarray(bq)[cs].reshape(128, 1).astype(np.float32),
            bk=np.asarray(bk)[cs].reshape(128, 1).astype(np.float32),
            bv=np.asarray(bv)[cs].reshape(128, 1).astype(np.float32),
            bo=(np.ascontiguousarray(
                    np.asarray(bo).astype(np.float32).reshape(KC, 128).T)
                if c == 0 else np.zeros((128, KC), np.float32)),
        ))
    return maps


def kernel(hidden_states, Wq, bq, Wk, bk, Wv, bv, Wo, bo):
    nc = _build()
    maps = _in_maps(hidden_states, Wq, bq, Wk, bk, Wv, bv, Wo, bo)
    res = run_bass_kernel_spmd(nc, maps, list(range(NCORES))).results
    acc = np.zeros((KC, 128, T), np.float64)
    for r in res:
        acc += r["po"]
    outT = acc.reshape(D, T)
    return outT.T.reshape(B, S, D).astype(np.float32)


# revision 19
# speedup vs baseline: 1.2330x; 1.2330x over previous
"""Multi-head causal attention (B=2, S=2048, H=16, DH=64, D=1024) on 8 TRN2 cores.

Sharding: Megatron tensor-parallel over heads — core c owns heads {2c, 2c+1}:
  * column-slices of Wq/Wk/Wv (128 cols each) + bias slices,
  * row-slice of Wo (128 rows),
  * full hidden_states (pre-transposed on host to [D, B*S]).
Each core computes a partial output (its 2 heads through Wo rows); host sums
the 8 partials (row-parallel unshard) — bo is added on device by core 0.

Device dataflow per core (all matmuls in float32r — full-rate fp32 with
15-bit mantissa — contract: every matmul operand must be produced by a
"rounding" instruction or DMA-ed from a float32r DRAM tensor):
  A) QKV: qT/kT/vT [128, 4096] = W_slice.T @ hiddenT  (contraction over D in
     8 chunks of 128; biases are per-partition adds during PSUM evacuation).
     vT is PE-transposed into V_aug [tok128, chunk, head, 65] with a ones
     column (65th) so the AV matmul also emits softmax denominators.
  B) Attention per (batch b, 256-query block qi), both heads:
     scoresT[kv,q] = kT_slice.T @ qT_slice  (K=64, N=256, two heads packed on
     PE row-groups 0-63/64-127), exp via ACT (scale=1/8) straight off PSUM,
     causal diag masked by a triangular multiply (even chunk) / restricting
     the AV matmul columns (odd chunk), then ctxT_aug[65, q] += V_aug.T@expT.
     Softmax denominators (row 64) are reciprocal-ed and broadcast across
     64 partitions with a K=1 ones matmul, then ctxT normalized via DVE.
  C) Output proj: outT[n,tok] = Wo_slice_chunk.T @ ctxT (+ bo via K=1 ones
     matmul, nonzero on core 0 only), DMA-ed to DRAM straight from PSUM.
"""
import os
import sys

sys.path.insert(0, "/opt/trn_rl_repo")

from contextlib import ExitStack

import numpy as np

import concourse.bass as bass
import concourse.mybir as mybir
import concourse.tile as tile
from concourse import bacc
from concourse.bass_utils import run_bass_kernel_spmd

F32 = mybir.dt.float32
F32R = mybir.dt.float32r
F16 = mybir.dt.float16
MM_DT = F32R if os.environ.get("KERNEL_F32R") == "1" else F16
MM_NP = np.float32 if os.environ.get("KERNEL_F32R") == "1" else np.float16

B, S, H, DH = 2, 2048, 16, 64
D = H * DH            # 1024
T = B * S             # 4096 tokens
NCORES = 8
HPC = H // NCORES     # 2 heads per core
KC = D // 128         # 8 contraction chunks
NG = T // 512         # 8 token groups for QKV
NQB = S // 256        # 8 query blocks per batch
NKV = T // 128        # 32 kv chunks of 128 tokens
EXPFN = mybir.ActivationFunctionType.Exp


def _body(nc, tc, ctx, t_in, t_out, t_out_dbg=None):
    xt, wq, wk, wv, wo, bq, bk, bv, bo, tri, ident, vones = t_in
    po = t_out

    const = ctx.enter_context(tc.tile_pool(name="const", bufs=1))
    big = ctx.enter_context(tc.tile_pool(name="big", bufs=1))
    xtp = ctx.enter_context(tc.tile_pool(name="xtp", bufs=3))
    vtp = ctx.enter_context(tc.tile_pool(name="vtp", bufs=2))
    ep = ctx.enter_context(tc.tile_pool(name="ep", bufs=6))
    rp = ctx.enter_context(tc.tile_pool(name="rp", bufs=2))
    rbp = ctx.enter_context(tc.tile_pool(name="rbp", bufs=2))

    psA = ctx.enter_context(tc.tile_pool(name="psA", bufs=2, space="PSUM"))
    psS = ctx.enter_context(tc.tile_pool(name="psS", bufs=2, space="PSUM"))
    psC = ctx.enter_context(tc.tile_pool(name="psC", bufs=1, space="PSUM"))

    # ---- constants / weights in SBUF
    wq_s = const.tile([128, KC, 128], MM_DT, tag="wq")
    wk_s = const.tile([128, KC, 128], MM_DT, tag="wk")
    wv_s = const.tile([128, KC, 128], MM_DT, tag="wv")
    wo_s = const.tile([128, D], MM_DT, tag="wo")
    bq_s = const.tile([128, 1], F32, tag="bq")
    bk_s = const.tile([128, 1], F32, tag="bk")
    bv_s = const.tile([128, 1], F32, tag="bv")
    bo_s = const.tile([128, KC], F32, tag="bo")
    tri_s = const.tile([128, 128], MM_DT, tag="tri")
    id_s = const.tile([128, 128], MM_DT, tag="ident")
    nc.sync.dma_start(wq_s[:], wq[:])
    nc.sync.dma_start(wk_s[:], wk[:])
    nc.sync.dma_start(wv_s[:], wv[:])
    nc.sync.dma_start(bq_s[:], bq[:])
    nc.sync.dma_start(bk_s[:], bk[:])
    nc.sync.dma_start(bv_s[:], bv[:])
    nc.gpsimd.dma_start(wo_s[:], wo[:])
    nc.gpsimd.dma_start(bo_s[:], bo[:])
    nc.gpsimd.dma_start(tri_s[:], tri[:])
    nc.gpsimd.dma_start(id_s[:], ident[:])

    qT = big.tile([128, T], MM_DT, tag="qT")
    kT = big.tile([128, T], MM_DT, tag="kT")
    vT = big.tile([128, T], MM_DT, tag="vT")
    ctxT = big.tile([128, T], MM_DT, tag="ctxT")
    vaug = big.tile([128, NKV, HPC, 65], MM_DT, tag="vaug")
    # ones column of V_aug (softmax denominator trick)
    nc.gpsimd.dma_start(vaug[:, :, :, 64].rearrange("p c h -> p (c h)"), vones[:])


    if t_out_dbg is not None:
        dbgp = ctx.enter_context(tc.tile_pool(name="dbgp", bufs=1))
        dbg_craw_s = dbgp.tile([128, T], F32, tag="craw")
        dbg_den_s = dbgp.tile([128, T], F32, tag="den")

    # ---- phase A: QKV projections + V transpose
    for g in range(NG):
        cols = slice(g * 512, (g + 1) * 512)
        xg = xtp.tile([128, KC, 512], MM_DT, tag="xt")
        nc.sync.dma_start(xg[:], xt[g])

        for (w_s, b_s, dst) in ((wq_s, bq_s, qT), (wk_s, bk_s, kT)):
            acc = psA.tile([128, 512], F32, tag="pqkv")
            for k in range(KC):
                nc.tensor.matmul(acc[:], w_s[:, k, :], xg[:, k, :],
                                 start=(k == 0), stop=(k == KC - 1))
            nc.vector.tensor_scalar_add(dst[:, cols], acc[:], b_s[:])

        acc = psA.tile([128, 512], F32, tag="pqkv")
        for k in range(KC):
            nc.tensor.matmul(acc[:], wv_s[:, k, :], xg[:, k, :],
                             start=(k == 0), stop=(k == KC - 1))
        nc.vector.tensor_scalar_add(vT[:, cols], acc[:], bv_s[:])

    # transpose vT into V_aug (natural [tok, dh] layout), batched after QKV
    for chunk in range(NKV):
        tp = psA.tile([128, 128], MM_DT, tag="pqkv")
        nc.tensor.transpose(tp[:], vT[:, chunk * 128:(chunk + 1) * 128],
                            id_s[:])
        nc.vector.tensor_copy(
            vaug[:, chunk, :, 0:64],
            tp[:].rearrange("p (h d) -> p h d", h=HPC))

    woc = wo_s[:].rearrange("p (c n) -> p c n", c=KC)
    osp = ctx.enter_context(tc.tile_pool(name="osp", bufs=6))

    # ---- phase B: attention, both heads, causal
    for qi in range(NQB):
        for b in range(B):
            qcols = slice(b * S + qi * 256, b * S + qi * 256 + 256)
            nch = 2 * (qi + 1)          # kv chunks of 128 for this q block
            ct0 = psC.tile([65, 256], F32, tag="ct0")
            ct1 = psC.tile([65, 256], F32, tag="ct1")
            cts = [ct0, ct1]
            nwaves = (nch + 1) // 2
            for w in range(nwaves):
                js = [j for j in (2 * w, 2 * w + 1) if j < nch]
                sp = psS.tile([128, 2 * HPC, 256], F32, tag="sp")
                for h in range(HPC):
                    for i, j in enumerate(js):
                        kcols = slice(b * S + j * 128, b * S + j * 128 + 128)
                        nc.tensor.matmul(
                            sp[:, 2 * h + i, :],
                            kT[h * 64:(h + 1) * 64, kcols],
                            qT[h * 64:(h + 1) * 64, qcols],
                            start=True, stop=True)
                e = ep.tile([128, 2 * HPC, 256], MM_DT, tag="e")
                nc.scalar.activation(e[:], sp[:], EXPFN, scale=0.125)
                for h in range(HPC):
                    for i, j in enumerate(js):
                        lhsT = vaug[:, b * (S // 128) + j, h, :]
                        first = (j == 0)
                        last = (j == nch - 1)
                        if j == nch - 2:  # even diag chunk: mask lower tri
                            nc.vector.tensor_mul(
                                e[:, 2 * h + i, 0:128],
                                e[:, 2 * h + i, 0:128], tri_s[:])
                            nc.tensor.matmul(cts[h][:, :], lhsT,
                                             e[:, 2 * h + i, :],
                                             start=first, stop=last)
                        elif j == nch - 1:  # odd diag: q first half all masked
                            nc.vector.tensor_mul(
                                e[:, 2 * h + i, 128:256],
                                e[:, 2 * h + i, 128:256], tri_s[:])
                            nc.tensor.matmul(cts[h][:, 128:256], lhsT,
                                             e[:, 2 * h + i, 128:256],
                                             start=first, stop=last)
                        else:
                            nc.tensor.matmul(cts[h][:, :], lhsT,
                                             e[:, 2 * h + i, :],
                                             start=first, stop=last)
            # normalize: ctxT[, q] = ct[0:64] * (1 / ct[64]) broadcast
            for h in range(HPC):
                sums = rp.tile([1, 256], F32, tag="sums")
                nc.vector.tensor_copy(sums[:], cts[h][64:65, :])
                r = rp.tile([1, 256], F32, tag="r")
                nc.vector.reciprocal_approx_fast(r[:], sums[:])
                rb = rbp.tile([64, 256], F32, tag="rb")
                nc.gpsimd.partition_broadcast(rb[:], r[:])
                nc.vector.tensor_mul(ctxT[h * 64:(h + 1) * 64, qcols],
                                     cts[h][0:64, :], rb[:])
                if t_out_dbg is not None:
                    nc.vector.tensor_copy(
                        dbg_craw_s[h * 64:(h + 1) * 64, qcols],
                        cts[h][0:64, :])
                    nc.vector.tensor_copy(
                        dbg_den_s[h * 64:(h + 1) * 64, qcols], rb[:])

            if qi % 2 == 1:
                # output projection for the completed 512-token block
                t = b * (S // 512) + qi // 2
                tcols = slice(t * 512, (t + 1) * 512)
                for c in range(KC):
                    op = psA.tile([128, 512], F32, tag="pqkv")
                    nc.tensor.matmul(op[:], woc[:, c, :], ctxT[:, tcols],
                                     start=True, stop=True)
                    ost = osp.tile([128, 512], F32, tag="ost")
                    if c % 2 == 0:
                        nc.vector.tensor_scalar_add(ost[:], op[:],
                                                    bo_s[:, c:c + 1])
                    else:
                        nc.scalar.activation(
                            ost[:], op[:],
                            mybir.ActivationFunctionType.Identity,
                            bias=bo_s[:, c:c + 1])
                    nc.sync.dma_start(po[c, :, tcols], ost[:])

    # ---- phase C: output projection (transposed out), bias, DMA from PSUM
    if t_out_dbg is not None:
        dq, dk, dc, dv, dcr, dde = t_out_dbg
        st = ctx.enter_context(tc.tile_pool(name="dbg", bufs=1))
        for src, dst in ((qT, dq), (kT, dk), (ctxT, dc)):
            tmp = st.tile([128, T], F32, tag="dbgt")
            nc.vector.tensor_copy(tmp[:], src[:])
            nc.sync.dma_start(dst[:], tmp[:])
        nc.sync.dma_start(dcr[:], dbg_craw_s[:])
        nc.sync.dma_start(dde[:], dbg_den_s[:])
        tmpv = st.tile([128, NKV * HPC * 65], F32, tag="dbgt")
        nc.vector.tensor_copy(
            tmpv[:], vaug[:].rearrange("p c h x -> p (c h x)"))
        nc.sync.dma_start(dv[:], tmpv[:])



_NC = None


def _build():
    global _NC
    if _NC is not None:
        return _NC
    nc = bacc.Bacc("TRN2", target_bir_lowering=False, debug=False,
                   num_devices=NCORES)
    t_in = [
        nc.dram_tensor("xt", [NG, 128, KC, 512], MM_DT, kind="ExternalInput").ap(),
        nc.dram_tensor("wq", [128, KC, 128], MM_DT, kind="ExternalInput").ap(),
        nc.dram_tensor("wk", [128, KC, 128], MM_DT, kind="ExternalInput").ap(),
        nc.dram_tensor("wv", [128, KC, 128], MM_DT, kind="ExternalInput").ap(),
        nc.dram_tensor("wo", [128, D], MM_DT, kind="ExternalInput").ap(),
        nc.dram_tensor("bq", [128, 1], F32, kind="ExternalInput").ap(),
        nc.dram_tensor("bk", [128, 1], F32, kind="ExternalInput").ap(),
        nc.dram_tensor("bv", [128, 1], F32, kind="ExternalInput").ap(),
        nc.dram_tensor("bo", [128, KC], F32, kind="ExternalInput").ap(),
        nc.dram_tensor("tri", [128, 128], MM_DT, kind="ExternalInput").ap(),
        nc.dram_tensor("ident", [128, 128], MM_DT, kind="ExternalInput").ap(),
        nc.dram_tensor("vones", [128, NKV * HPC], MM_DT,
                       kind="ExternalInput").ap(),
    ]
    po = nc.dram_tensor("po", [KC, 128, T], F32, kind="ExternalOutput").ap()
    t_out_dbg = None
    if os.environ.get("KERNEL_DEBUG_TAPS") == "1":
        t_out_dbg = [
            nc.dram_tensor("dbg_qT", [128, T], F32, kind="ExternalOutput").ap(),
            nc.dram_tensor("dbg_kT", [128, T], F32, kind="ExternalOutput").ap(),
            nc.dram_tensor("dbg_ctxT", [128, T], F32, kind="ExternalOutput").ap(),
            nc.dram_tensor("dbg_vaug", [128, NKV * HPC * 65], F32,
                           kind="ExternalOutput").ap(),
            nc.dram_tensor("dbg_craw", [128, T], F32, kind="ExternalOutput").ap(),
            nc.dram_tensor("dbg_den", [128, T], F32, kind="ExternalOutput").ap(),
        ]
    with tile.TileContext(nc) as tc, ExitStack() as ctx:
        _body(nc, tc, ctx, t_in, po, t_out_dbg)
    nc.compile()
    _NC = nc
    return nc


def _in_maps(hidden_states, Wq, bq, Wk, bk, Wv, bv, Wo, bo):
    hid = np.asarray(hidden_states, dtype=np.float32).reshape(T, D)
    hidT = hid.T.astype(MM_NP)                       # [D, T]
    xt = np.ascontiguousarray(
        hidT.reshape(KC, 128, NG, 512).transpose(2, 1, 0, 3))
    common = {
        "xt": xt,
        "tri": np.triu(np.ones((128, 128), MM_NP)),
        "ident": np.eye(128, dtype=MM_NP),
        "vones": np.ones((128, NKV * HPC), MM_NP),
    }
    maps = []
    for c in range(NCORES):
        cs = slice(c * 128, (c + 1) * 128)
        maps.append(dict(
            common,
            wq=np.ascontiguousarray(np.asarray(Wq)[:, cs].astype(MM_NP).reshape(KC, 128, 128).transpose(1, 0, 2)),
            wk=np.ascontiguousarray(np.asarray(Wk)[:, cs].astype(MM_NP).reshape(KC, 128, 128).transpose(1, 0, 2)),
            wv=np.ascontiguousarray(np.asarray(Wv)[:, cs].astype(MM_NP).reshape(KC, 128, 128).transpose(1, 0, 2)),
            wo=np.ascontiguousarray(np.asarray(Wo)[cs, :].astype(MM_NP)),
            bq=np.asarray(bq)[cs].reshape(128, 1).astype(np.float32),
            bk=np.asarray(bk)[cs].reshape(128, 1).astype(np.float32),
            bv=np.asarray(bv)[cs].reshape(128, 1).astype(np.float32),
            bo=(np.ascontiguousarray(
                    np.asarray(bo).astype(np.float32).reshape(KC, 128).T)
                if c == 0 else np.zeros((128, KC), np.float32)),
        ))
    return maps


def kernel(hidden_states, Wq, bq, Wk, bk, Wv, bv, Wo, bo):
    nc = _build()
    maps = _in_maps(hidden_states, Wq, bq, Wk, bk, Wv, bv, Wo, bo)
    res = run_bass_kernel_spmd(nc, maps, list(range(NCORES))).results
    acc = np.zeros((KC, 128, T), np.float64)
    for r in res:
        acc += r["po"]
    outT = acc.reshape(D, T)
    return outT.T.reshape(B, S, D).astype(np.float32)
